# revision 39
# baseline (speedup 1.0000x reference)
# Trainium2 Bass kernel for a single pre-norm transformer block
# (LN1 -> 6-head causal self-attention -> residual -> LN2 -> 1536-wide relu MLP -> residual).
#
# Sharding: pure data-parallel over batch. B=128 sequences split 16-per-core
# across 8 NeuronCores; weights are replicated; no collectives.
#
# Design (v19):
#   - All weight preprocessing on HOST (numpy): LN gammas folded into the
#     bf16/fp8 weights, LN betas folded into fused biases (bq/bk,
#     bo2 = bo + (ln1_b@Wv)@Wo, b1' = b1 + ln2_b@W1), layouts pre-transposed
#     feature-major, constants (identity/causal-mask/one-hot/indicator)
#     shipped as inputs. The device kernel has NO preamble.
#   - Activations "feature-major" (FM): [C partitions (3x128 chunks), tokens];
#     chained matmuls need no transposes. LN runs token-major (bn_stats over
#     free dim), normalized tile PE-transposed into FM.
#   - Softmax (transposed scores, no max-subtraction):
#     denominators = one-hot-column-stationary matmuls that partition-reduce
#     each masked expT tile into row h of a [6, 384] psum group; the causal
#     mask for all 6 heads of a seq is ONE contiguous DVE multiply
#     ([triu|ones|triu] row broadcast over heads); NEGATED reciprocal via a
#     bit-trick seed + 1 Newton step (z' = (d*z+2)*z); per-head-pair
#     broadcast = ONE (-1)-indicator-stationary matmul per hp that
#     overwrites the drained attnV psum; DVE multiplies sbuf x psum.
#   - FFN entirely fp8 (e4m3) DoubleRow, with both moving operands
#     pair-interleaved (host-interleaved W2 [P, jp, C, 2]; h2 stored as
#     [P, TS, 2] pair + [P, TS] single) so DR streams byte-adjacent rows.
#   - Depth-2 software pipeline: iteration s runs supertile s and preps
#     s+2 (x DMA, LN1 chain, qkv) in its tail where DVE/Act have slack;
#     next supertile's score blocks split per-seq around the LN2 transposes
#     as PE filler; engine assignment tuned per-op (exps/yn/ar/qk-bias on
#     Act, stats/copies/divisions/z-odd on DVE, residual pre-adds on
#     GPSIMD, weights+consts DMA on the gpsimd queue, x/out on sync).
#
# Measured (NTFF profile, core 0): ~350.6us span per pass vs 471.5us for
# the session-start baseline; rel err 1.40e-2 (gate 2e-2).

import numpy as np

P = 128
B, T, C, H, D = 128, 256, 384, 6, 64
NCORES = 8
B_LOC = B // NCORES          # 16 sequences per core
NTOK = B_LOC * T             # 4096 tokens per core
TS = 2 * T                   # 512-token supertile = 2 sequences
NSUP = NTOK // TS            # 8
CJ = C // P                  # 3 chunks of the 384 model dim
FF = 4 * C                   # 1536
FJ = FF // P                 # 12 chunks of the FFN hidden dim
NTT = TS // P                # 4 token tiles per supertile
EPS = 1e-5
SCALE = D ** (-0.5)

_CACHE = {}


def _build_nc(niter=1):
    import concourse.bass as bass
    import concourse.tile as tile
    from concourse import bacc, mybir
    from contextlib import ExitStack

    F32 = mybir.dt.float32
    BF16 = mybir.dt.bfloat16
    FP8 = mybir.dt.float8e4
    DR = mybir.MatmulPerfMode.DoubleRow

    nc = bacc.Bacc("TRN2", target_bir_lowering=False, debug=False,
                   num_devices=NCORES)

    x_d = nc.dram_tensor("x", [NTOK, C], F32, kind="ExternalInput").ap()
    wq_d = nc.dram_tensor("wq", [P, CJ, C], BF16, kind="ExternalInput").ap()
    wk_d = nc.dram_tensor("wk", [P, CJ, C], BF16, kind="ExternalInput").ap()
    wv_d = nc.dram_tensor("wv", [P, CJ, C], BF16, kind="ExternalInput").ap()
    wo_d = nc.dram_tensor("wo", [P, CJ, C], BF16, kind="ExternalInput").ap()
    w1_d = nc.dram_tensor("w1", [P, CJ, FF], FP8, kind="ExternalInput").ap()
    # w2 pre-interleaved on host: [P, pair jp, C, 2] so the DoubleRow moving
    # operand reads byte-adjacent chunk pairs (full 2x fp8 stream rate)
    w2_d = nc.dram_tensor("w2", [P, FJ // 2, C, 2], FP8, kind="ExternalInput").ap()
    bq_d = nc.dram_tensor("bq", [P, CJ], F32, kind="ExternalInput").ap()
    bk_d = nc.dram_tensor("bk", [P, CJ], F32, kind="ExternalInput").ap()
    b1p_d = nc.dram_tensor("b1p", [P, FJ], F32, kind="ExternalInput").ap()
    bo2_d = nc.dram_tensor("bo2", [P, C], F32, kind="ExternalInput").ap()
    b2t_d = nc.dram_tensor("b2t", [P, C], F32, kind="ExternalInput").ap()
    ident_d = nc.dram_tensor("identb", [P, P], BF16, kind="ExternalInput").ap()
    mask_d = nc.dram_tensor("maskb", [P, CJ * P], BF16, kind="ExternalInput").ap()
    ones_d = nc.dram_tensor("onesc", [P, H, H], BF16, kind="ExternalInput").ap()
    ind_d = nc.dram_tensor("indb", [H, CJ * P], BF16, kind="ExternalInput").ap()
    out_d = nc.dram_tensor("out", [NTOK, C], F32, kind="ExternalOutput").ap()

    Exp = mybir.ActivationFunctionType.Exp
    Relu = mybir.ActivationFunctionType.Relu
    Ident = mybir.ActivationFunctionType.Identity
    I32 = mybir.dt.int32
    ADD = mybir.AluOpType.add
    MULT = mybir.AluOpType.mult
    MAX = mybir.AluOpType.max
    SHR = mybir.AluOpType.logical_shift_right

    with tile.TileContext(nc) as tc, ExitStack() as ctx:
        consts = ctx.enter_context(tc.tile_pool(name="consts", bufs=1))
        wpool = ctx.enter_context(tc.tile_pool(name="weights", bufs=1))
        xpool = ctx.enter_context(tc.tile_pool(name="xln", bufs=12))
        ps_big = ctx.enter_context(tc.tile_pool(name="psbig", bufs=3, space="PSUM"))
        ps_tr = ctx.enter_context(tc.tile_pool(name="pstr", bufs=2, space="PSUM"))
        ps_dn = ctx.enter_context(tc.tile_pool(name="psdn", bufs=1, space="PSUM"))
        ps_at = ctx.enter_context(tc.tile_pool(name="psat", bufs=2, space="PSUM"))

        # ----------- constants + weights: DMAs on the GPSIMD queue ------
        # (x loads / out stores own the Sync queue; Act queue stays clean
        # for the first LN normalize; gpsimd is idle until mid-supertile-0
        # so the ~15 descriptor issues cost nothing)
        ident_bf = consts.tile([P, P], BF16, tag="identbf")
        nc.gpsimd.dma_start(ident_bf[:], ident_d)
        maskf = consts.tile([P, CJ * P], BF16, tag="mask")
        nc.gpsimd.dma_start(maskf[:], mask_d)
        # onesh[:, h, :] is the one-hot-column stationary that routes head
        # h's partition-reduction into psum row h (matmul psum base must be
        # 0/32/64, so all heads accumulate into one base-0 [H, T] group)
        onesh = consts.tile([P, H, H], BF16, tag="onesh")
        nc.gpsimd.dma_start(onesh[:], ones_d)
        ind6 = consts.tile([H, CJ * P], BF16, tag="ind6")
        nc.gpsimd.dma_start(ind6[:], ind_d)
        bq = consts.tile([P, CJ], F32, tag="bq")
        nc.gpsimd.dma_start(bq[:], bq_d)
        bk = consts.tile([P, CJ], F32, tag="bk")
        nc.gpsimd.dma_start(bk[:], bk_d)
        b1p = consts.tile([P, FJ], F32, tag="b1p")
        nc.gpsimd.dma_start(b1p[:], b1p_d)
        bo2_bc = consts.tile([P, C], F32, tag="bo2_bc")
        nc.gpsimd.dma_start(bo2_bc[:], bo2_d)
        b2_bc = consts.tile([P, C], F32, tag="b2_bc")
        nc.gpsimd.dma_start(b2_bc[:], b2t_d)

        wq = wpool.tile([P, CJ, C], BF16, tag="wqb")
        nc.gpsimd.dma_start(wq[:], wq_d)
        wk = wpool.tile([P, CJ, C], BF16, tag="wkb")
        nc.gpsimd.dma_start(wk[:], wk_d)
        wv = wpool.tile([P, CJ, C], BF16, tag="wvb")
        nc.gpsimd.dma_start(wv[:], wv_d)
        wo = wpool.tile([P, CJ, C], BF16, tag="wob")
        nc.gpsimd.dma_start(wo[:], wo_d)
        w1 = wpool.tile([P, CJ, FF], FP8, tag="w1b")
        nc.gpsimd.dma_start(w1[:], w1_d)
        w2 = wpool.tile([P, FJ // 2, C, 2], FP8, tag="w2b")
        nc.gpsimd.dma_start(w2[:], w2_d)

        def load_x(s):
            tok0 = s * TS
            x_ts = []
            for ti in range(NTT):
                x_t = xpool.tile([P, C], F32, tag="x")
                nc.sync.dma_start(
                    x_t[:], x_d[tok0 + ti * P: tok0 + (ti + 1) * P, :])
                x_ts.append(x_t)
            return x_ts

        # ---------------- layernorm helpers ----------------
        spool = ctx.enter_context(tc.tile_pool(name="stats", bufs=6))
        ynpool = ctx.enter_context(tc.tile_pool(name="yn", bufs=8))

        def ln_stats_pre(src_tiles):
            """bn_stats per tile (each depends only on its x DMA)."""
            sts = []
            for ti in range(NTT):
                st = spool.tile([P, 6], F32, tag="bn")
                nc.vector.bn_stats(st[:], src_tiles[ti][:])
                sts.append(st)
            return sts

        def ln_stats_post(sts):
            """Aggregate + rstd Newton chain (emitted later so latency-
            critical attention DVE ops aren't queued behind it)."""
            mv4 = spool.tile([P, NTT, 2], F32, tag="mv")
            rstd4 = spool.tile([P, NTT], F32, tag="rstd")
            for ti in range(NTT):
                nc.vector.bn_aggr(mv4[:, ti, :], sts[ti][:])
            # rstd = rsqrt(var + eps): int32 seed + 2 Newton steps (no tables)
            veps = spool.tile([P, NTT], F32, tag="veps")
            nc.vector.tensor_scalar_add(veps[:], mv4[:, :, 1], EPS)
            iv = spool.tile([P, NTT], I32, tag="ivh")
            nc.vector.tensor_scalar(iv[:], veps[:].bitcast(I32), 1, None, op0=SHR)
            nc.vector.tensor_scalar(iv[:], iv[:], -1, 0x5F3759DF, op0=MULT, op1=ADD)
            tn = spool.tile([P, NTT], F32, tag="tnh")
            yv = iv[:].bitcast(F32)
            # 1 Newton step: ~0.1% rstd error, well under the bf16 yn output
            for it in range(1):
                nc.vector.tensor_tensor(tn[:], yv, yv, op=MULT)
                nc.vector.scalar_tensor_tensor(tn[:], tn[:], -0.5, veps[:],
                                               op0=MULT, op1=MULT)
                nc.vector.scalar_tensor_tensor(yv, tn[:], 1.5, yv,
                                               op0=ADD, op1=MULT)
            nc.vector.tensor_copy(rstd4[:], yv)
            nbias = spool.tile([P, NTT], F32, tag="nb")
            nc.vector.scalar_tensor_tensor(nbias[:], mv4[:, :, 0], -1.0,
                                           rstd4[:], op0=MULT, op1=MULT)
            return rstd4, nbias

        def ln_apply(src_tiles, stats, dst_fm):
            """Normalize token-major (Act) + PE transpose to FM + DVE copy."""
            rstd4, nbias = stats
            for ti in range(NTT):
                yn = ynpool.tile([P, C], BF16, tag="yn")
                nc.scalar.activation(yn[:], src_tiles[ti][:], Ident,
                                     bias=nbias[:, ti:ti + 1],
                                     scale=rstd4[:, ti:ti + 1])
                pst = ps_tr.tile([P, C], BF16, tag="tr")
                for j in range(CJ):
                    nc.tensor.transpose(pst[:, j * P:(j + 1) * P],
                                        yn[:, j * P:(j + 1) * P], ident_bf[:])
                nc.vector.tensor_copy(
                    dst_fm[:, :, ti * P:(ti + 1) * P],
                    pst[:].rearrange("p (j t) -> p j t", j=CJ))

        def ln_apply_pair(src_tiles, stats, dstp, dsts):
            """Like ln_apply but writes chunks 0,1 pair-interleaved
            ([P, TS, 2]) + chunk 2 separate, so the FFN1 DoubleRow moving
            operand reads byte-adjacent chunk pairs."""
            rstd4, nbias = stats
            for ti in range(NTT):
                yn = ynpool.tile([P, C], BF16, tag="yn")
                nc.scalar.activation(yn[:], src_tiles[ti][:], Ident,
                                     bias=nbias[:, ti:ti + 1],
                                     scale=rstd4[:, ti:ti + 1])
                pst = ps_tr.tile([P, C], BF16, tag="tr")
                for j in range(CJ):
                    nc.tensor.transpose(pst[:, j * P:(j + 1) * P],
                                        yn[:, j * P:(j + 1) * P], ident_bf[:])
                nc.vector.tensor_copy(
                    dstp[:, ti * P:(ti + 1) * P, :],
                    pst[:, 0:2 * P].rearrange("p (two t) -> p t two", two=2))
                nc.vector.tensor_copy(dsts[:, ti * P:(ti + 1) * P],
                                      pst[:, 2 * P:3 * P])

        # ---------------- pools for the main phases ----------------
        hpool = ctx.enter_context(tc.tile_pool(name="hfm", bufs=2))
        h2pool = ctx.enter_context(tc.tile_pool(name="h2fm", bufs=2))
        qkpool = ctx.enter_context(tc.tile_pool(name="qk", bufs=6))
        vpool = ctx.enter_context(tc.tile_pool(name="vton", bufs=12))
        xbpool = ctx.enter_context(tc.tile_pool(name="xbo", bufs=6))
        o1pool = ctx.enter_context(tc.tile_pool(name="o1res", bufs=6))
        obpool = ctx.enter_context(tc.tile_pool(name="o1b2", bufs=6))
        apool = ctx.enter_context(tc.tile_pool(name="attnfm", bufs=2))
        epool = ctx.enter_context(tc.tile_pool(name="expT", bufs=4))
        arpool = ctx.enter_context(tc.tile_pool(name="attnraw", bufs=8))
        zpool = ctx.enter_context(tc.tile_pool(name="zfm", bufs=1))
        ypool = ctx.enter_context(tc.tile_pool(name="yout", bufs=3))

        def ln_stats(src_tiles):
            return ln_stats_post(ln_stats_pre(src_tiles))

        def ln1_first(x_ts):
            """Supertile-0 LN1 with per-tile serial chains: tile 0's
            normalize/transpose starts as soon as ITS stats are done
            instead of after all four bn_stats (startup critical path)."""
            h_fm = hpool.tile([P, CJ, TS], BF16, tag="hfm")
            for ti in range(NTT):
                st = spool.tile([P, 6], F32, tag="bn")
                nc.vector.bn_stats(st[:], x_ts[ti][:])
                mv = spool.tile([P, 2], F32, tag="mv1")
                nc.vector.bn_aggr(mv[:], st[:])
                veps = spool.tile([P, 1], F32, tag="veps1")
                nc.vector.tensor_scalar_add(veps[:], mv[:, 1:2], EPS)
                iv = spool.tile([P, 1], I32, tag="iv1")
                nc.vector.tensor_scalar(iv[:], veps[:].bitcast(I32), 1, None,
                                        op0=SHR)
                nc.vector.tensor_scalar(iv[:], iv[:], -1, 0x5F3759DF,
                                        op0=MULT, op1=ADD)
                tn = spool.tile([P, 1], F32, tag="tn1")
                yv = iv[:].bitcast(F32)
                nc.vector.tensor_tensor(tn[:], yv, yv, op=MULT)
                nc.vector.scalar_tensor_tensor(tn[:], tn[:], -0.5, veps[:],
                                               op0=MULT, op1=MULT)
                nc.vector.scalar_tensor_tensor(yv, tn[:], 1.5, yv,
                                               op0=ADD, op1=MULT)
                nbias = spool.tile([P, 1], F32, tag="nb1")
                nc.vector.scalar_tensor_tensor(nbias[:], mv[:, 0:1], -1.0,
                                               yv, op0=MULT, op1=MULT)
                yn = ynpool.tile([P, C], BF16, tag="yn")
                nc.scalar.activation(yn[:], x_ts[ti][:], Ident,
                                     bias=nbias[:], scale=yv)
                pst = ps_tr.tile([P, C], BF16, tag="tr")
                for j in range(CJ):
                    nc.tensor.transpose(pst[:, j * P:(j + 1) * P],
                                        yn[:, j * P:(j + 1) * P], ident_bf[:])
                nc.vector.tensor_copy(
                    h_fm[:, :, ti * P:(ti + 1) * P],
                    pst[:].rearrange("p (j t) -> p j t", j=CJ))
            return h_fm

        def qkv_phase(h_fm):
            q_fm = qkpool.tile([P, CJ, TS], BF16, tag="qk")
            k_fm = qkpool.tile([P, CJ, TS], BF16, tag="qk")
            for wt, bt, dst in ((wq, bq, q_fm), (wk, bk, k_fm)):
                for f in range(CJ):
                    ps = ps_big.tile([P, TS], F32, tag="big")
                    for j in range(CJ):
                        nc.tensor.matmul(
                            ps[:], lhsT=wt[:, j, f * P:(f + 1) * P],
                            rhs=h_fm[:, j, :],
                            start=(j == 0), stop=(j == CJ - 1))
                    nc.scalar.activation(dst[:, f, :], ps[:], Ident,
                                         bias=bt[:, f:f + 1])
            v_ts = []
            for ti in range(NTT):
                ps = ps_big.tile([P, C], F32, tag="big")
                for j in range(CJ):
                    nc.tensor.matmul(
                        ps[:], lhsT=h_fm[:, j, ti * P:(ti + 1) * P],
                        rhs=wv[:, j, :],
                        start=(j == 0), stop=(j == CJ - 1))
                v_t = vpool.tile([P, C], BF16, tag="v")
                # DVE, not Act: the Act queue is the constraint in the
                # qkv->ln2 window
                nc.vector.tensor_copy(v_t[:], ps[:])
                v_ts.append(v_t)
            return q_fm, k_fm, v_ts

        def score_block(q_fm, k_fm, e_seq, seq, h):
            """Scores + exp for one (seq, head) block into e_seq[:, h, :].
            narrow layout: cols 0:256 = [k 0:128 x q 0:256], cols 256:384 =
            [k 128:256 x q 128:256]."""
            t0 = seq * T
            hp, hh = h // 2, h % 2
            pr = slice(hh * D, (hh + 1) * D)
            ps_sc = ps_big.tile([P, 3 * P], F32, tag="big")
            nc.tensor.matmul(ps_sc[:, 0:T],
                             lhsT=k_fm[pr, hp, t0:t0 + P],
                             rhs=q_fm[pr, hp, t0:t0 + T],
                             start=True, stop=True)
            nc.tensor.matmul(ps_sc[:, T:T + P],
                             lhsT=k_fm[pr, hp, t0 + P:t0 + T],
                             rhs=q_fm[pr, hp, t0 + P:t0 + T],
                             start=True, stop=True)
            nc.scalar.activation(e_seq[:, h, :], ps_sc[:], Exp, scale=SCALE)

        def mask_seq(e_seq):
            """Mask all 6 heads in one contiguous-inner-dim DVE multiply:
            maskf = [triu | ones | triu] broadcast over heads via a
            0-stride dim (contiguous 384-wide rows keep DVE at full rate;
            the middle block multiplies by 1)."""
            ev = e_seq[:]
            mk = maskf[:]
            mbc = bass.AP(tensor=mk.tensor, offset=mk.offset,
                          ap=[list(mk.ap[0]), [0, H], [1, CJ * P]])
            nc.vector.tensor_tensor(ev, ev, mbc, op=MULT)

        def attn_scores_seq(q_fm, k_fm, seq):
            """Scores+exp+mask for the 6 head blocks of one seq."""
            e_seq = epool.tile([P, H, 3 * P], BF16, tag="e")
            for h in range(H):
                score_block(q_fm, k_fm, e_seq, seq, h)
            mask_seq(e_seq)
            return [e_seq[:, h, :] for h in range(H)]

        def attn_scores(q_fm, k_fm):
            return [attn_scores_seq(q_fm, k_fm, 0),
                    attn_scores_seq(q_fm, k_fm, 1)]

        def attention_seq(exps_2, v_ts, attn_fm, seq):
                t0 = seq * T
                v0, v1 = v_ts[2 * seq], v_ts[2 * seq + 1]
                exps = exps_2[seq]  # noqa: kept names for the body below
                # denominators: partition-reduce each masked expT into row h
                # of dn via the one-hot-column stationary (rows != h get +0);
                # all 6 matmuls accumulate into one base-0 psum group.
                # dn cols 0:256 = keys 0:128 over q 0:256; cols 256:384 =
                # keys 128:256 over q 128:256 (folded below on DVE).
                dn = ps_dn.tile([H, 3 * P], F32, tag="dn")
                for h in range(H):
                    nc.tensor.matmul(dn[:],
                                     lhsT=onesh[:, h, :], rhs=exps[h][:],
                                     start=(h == 0), stop=(h == H - 1))
                # fold + NEGATED reciprocal via bit-trick seed + 1 Newton
                # step in z = -1/d space (z' = (d*z + 2)*z); the indicator
                # matrix carries -1 entries so the broadcast flips the sign.
                dsb = spool.tile([H, 3 * P], F32, tag="dsb")
                nc.vector.tensor_copy(dsb[:], dn[:])
                nc.vector.tensor_tensor(dsb[:, P:T], dsb[:, P:T],
                                        dsb[:, T:3 * P], op=ADD)
                zi = spool.tile([H, T], I32, tag="zi")
                nc.vector.tensor_scalar(zi[:], dsb[:, 0:T].bitcast(I32),
                                        -1, 0xFEF311C3 - (1 << 32),
                                        op0=MULT, op1=ADD)
                zf = zi[:].bitcast(F32)
                tn2 = spool.tile([H, T], F32, tag="tn2")
                nc.vector.tensor_tensor(tn2[:], dsb[:, 0:T], zf, op=MULT)
                recip = spool.tile([H, T], BF16, tag="recip")
                with nc.allow_low_precision(reason="bf16 softmax recip"):
                    nc.vector.scalar_tensor_tensor(recip[:], tn2[:], 2.0, zf,
                                                   op0=ADD, op1=MULT)
                # attnV: two [P, 512] psum tiles hold hp0|hp1 and hp2|M0;
                # M1/M2 overwrite the drained hp0/hp1 regions.
                psA = ps_at.tile([P, TS], F32, tag="at")
                psB = ps_at.tile([P, TS], F32, tag="at")
                regions = [(psA, 0), (psA, T), (psB, 0)]
                mregions = [(psB, T), (psA, 0), (psA, T)]
                ars = []
                for hp in range(CJ):
                    ps_a, c0 = regions[hp]
                    for hh in range(2):
                        h = 2 * hp + hh
                        po = slice(hh * D, (hh + 1) * D)
                        nc.tensor.matmul(ps_a[po, c0:c0 + T],
                                         lhsT=v0[:, h * D:(h + 1) * D],
                                         rhs=exps[h][:, 0:T],
                                         start=True, stop=False)
                        nc.tensor.matmul(ps_a[po, c0 + P:c0 + T],
                                         lhsT=v1[:, h * D:(h + 1) * D],
                                         rhs=exps[h][:, T:T + P],
                                         start=False, stop=True)
                    ar = arpool.tile([P, T], BF16, tag="ar")
                    # Act, not DVE: Act idles during attention and this
                    # keeps the psum-drain chain off the loaded DVE queue
                    nc.scalar.activation(ar[:], ps_a[:, c0:c0 + T], Ident)
                    ars.append(ar)
                # recip row-broadcast per head-pair psum block, then divide
                for hp in range(CJ):
                    ps_m, m0 = mregions[hp]
                    nc.tensor.matmul(ps_m[:, m0:m0 + T],
                                     lhsT=ind6[:, hp * P:(hp + 1) * P],
                                     rhs=recip[:], start=True, stop=True)
                    nc.vector.tensor_tensor(attn_fm[:, hp, t0:t0 + T],
                                            ars[hp][:], ps_m[:, m0:m0 + T],
                                            op=MULT)

        def wo_tile(attn_fm, x_ts, ti, o1_ts, ob_ts):
            xb = xbpool.tile([P, C], F32, tag="xb")
            nc.gpsimd.tensor_tensor(xb[:], x_ts[ti][:], bo2_bc[:], op=ADD)
            # ps_at (free after attention), so qk never waits on wo drains
            ps = ps_at.tile([P, C], F32, tag="at")
            for j in range(CJ):
                nc.tensor.matmul(
                    ps[:], lhsT=attn_fm[:, j, ti * P:(ti + 1) * P],
                    rhs=wo[:, j, :],
                    start=(j == 0), stop=(j == CJ - 1))
            o1 = o1pool.tile([P, C], F32, tag="o1")
            nc.vector.tensor_tensor(o1[:], ps[:], xb[:], op=ADD)
            o1_ts.append(o1)
            ob = obpool.tile([P, C], F32, tag="ob")
            nc.gpsimd.tensor_tensor(ob[:], o1[:], b2_bc[:], op=ADD)
            ob_ts.append(ob)

        def attn_wo_phase(exps_2, v_ts, x_ts):
            attn_fm = apool.tile([P, CJ, TS], BF16, tag="attn")
            o1_ts, ob_ts, sts = [], [], []
            attention_seq(exps_2, v_ts, attn_fm, 0)
            attention_seq(exps_2, v_ts, attn_fm, 1)
            for ti in range(NTT):
                wo_tile(attn_fm, x_ts, ti, o1_ts, ob_ts)
                # LN2 stats per tile right behind its o1 add: shortens the
                # serial o1 -> stats -> yn -> transpose chain
                st = spool.tile([P, 6], F32, tag="bn")
                nc.vector.bn_stats(st[:], o1_ts[ti][:])
                sts.append(st)
            return o1_ts, ob_ts, sts

        def ffn_phase(h2p, h2s, ob_ts, tok0, qk_next=None):
            """FFN for supertile s; when qk_next is given, the NEXT
            supertile's 12 score blocks interleave into the f-chunk loop
            (one per chunk) so their exps stream on Act alongside the z
            writes instead of piling up after them."""
            exps_next = None
            e_cur = None
            if qk_next is not None:
                q_fm, k_fm = qk_next
                exps_next = []
            z_fm = zpool.tile([P, FJ, TS], FP8, tag="z")
            h2pv = h2p[:].rearrange("p t two -> p two t")
            for f in range(FJ):
                ps = ps_big.tile([P, TS], F32, tag="big")
                if f < 2:
                    # token-split: these start after only 2 h2 tiles land,
                    # instead of waiting for the whole supertile's copies
                    for half in range(2):
                        sl = slice(half * T, (half + 1) * T)
                        nc.tensor.matmul(
                            ps[:, sl], lhsT=w1[:, 0:2, f * P:(f + 1) * P],
                            rhs=h2pv[:, :, sl], perf_mode=DR,
                            start=True, stop=False)
                        nc.tensor.matmul(
                            ps[:, sl], lhsT=w1[:, 2, f * P:(f + 1) * P],
                            rhs=h2s[:, sl],
                            start=False, stop=True)
                else:
                    nc.tensor.matmul(
                        ps[:], lhsT=w1[:, 0:2, f * P:(f + 1) * P],
                        rhs=h2pv, perf_mode=DR,
                        start=True, stop=False)
                    nc.tensor.matmul(
                        ps[:], lhsT=w1[:, 2, f * P:(f + 1) * P],
                        rhs=h2s[:],
                        start=False, stop=True)
                if f % 2 == 0:
                    nc.scalar.activation(z_fm[:, f, :], ps[:], Relu,
                                         bias=b1p[:, f:f + 1])
                else:
                    nc.vector.tensor_scalar(z_fm[:, f, :], ps[:],
                                            b1p[:, f:f + 1], 0.0,
                                            op0=ADD, op1=MAX)
                if qk_next is not None:
                    seq, h = divmod(f, H)
                    if h == 0:
                        e_cur = epool.tile([P, H, 3 * P], BF16, tag="e")
                    score_block(q_fm, k_fm, e_cur, seq, h)
                    if h == H - 1:
                        mask_seq(e_cur)
                        exps_next.append([e_cur[:, h2, :] for h2 in range(H)])
            for ti in range(NTT):
                ps = ps_big.tile([P, C], F32, tag="big")
                for jp in range(FJ // 2):
                    nc.tensor.matmul(
                        ps[:], lhsT=z_fm[:, 2 * jp:2 * jp + 2,
                                         ti * P:(ti + 1) * P],
                        rhs=w2[:, jp].rearrange("p c two -> p two c"),
                        perf_mode=DR,
                        start=(jp == 0), stop=(jp == FJ // 2 - 1))
                y_t = ypool.tile([P, C], F32, tag="y")
                nc.vector.tensor_tensor(y_t[:], ps[:], ob_ts[ti][:], op=ADD)
                nc.sync.dma_start(
                    out_d[tok0 + ti * P: tok0 + (ti + 1) * P, :], y_t[:])
            return exps_next

        def main_pass(_iv=None):
            # depth-2 software pipeline: iteration s runs the block for
            # supertile s and PREPS supertile s+2 (x DMA, LN1 chain, qkv) in
            # its tail, where DVE/Act have slack -- the serial LN1 chain is
            # never on the critical path. exps(s+1) also stream at the tail.
            x_pre = {0: load_x(0)}
            h0 = ln1_first(x_pre[0])
            qkv_pre = {0: qkv_phase(h0)}
            x_pre[1] = load_x(1)
            h1 = hpool.tile([P, CJ, TS], BF16, tag="hfm")
            ln_apply(x_pre[1], ln_stats(x_pre[1]), h1)
            qkv_pre[1] = qkv_phase(h1)
            exps_cur = attn_scores(qkv_pre[0][0], qkv_pre[0][1])
            for s in range(NSUP):
                o1_ts, ob_ts, sts2 = attn_wo_phase(exps_cur, qkv_pre[s][2],
                                                   x_pre[s])
                st2 = ln_stats_post(sts2)
                if s + 1 < NSUP:
                    # seq0 of the next supertile's scores: PE filler between
                    # wo and the LN2 transposes (hides the serial LN2 chain)
                    e0_next = attn_scores_seq(qkv_pre[s + 1][0],
                                              qkv_pre[s + 1][1], 0)
                h2p = h2pool.tile([P, TS, 2], FP8, tag="h2p")
                h2s = h2pool.tile([P, TS], FP8, tag="h2s")
                ln_apply_pair(o1_ts, st2, h2p, h2s)
                if s + 1 < NSUP:
                    # seq1 scores: PE filler between the LN2 transposes and
                    # ffn1 (hides the h2 psum->sbuf copy latency)
                    e1_next = attn_scores_seq(qkv_pre[s + 1][0],
                                              qkv_pre[s + 1][1], 1)
                    exps_cur = [e0_next, e1_next]
                ffn_phase(h2p, h2s, ob_ts, s * TS)
                if s + 2 < NSUP:
                    x_pre[s + 2] = load_x(s + 2)
                    st = ln_stats(x_pre[s + 2])
                    h_n = hpool.tile([P, CJ, TS], BF16, tag="hfm")
                    ln_apply(x_pre[s + 2], st, h_n)
                    qkv_pre[s + 2] = qkv_phase(h_n)
                    del x_pre[s], qkv_pre[s]

        if niter == 1:
            main_pass()
        else:
            with tc.For_i(0, niter, 1) as iv:
                main_pass(iv)

    nc.compile()
    return nc


def _build_runner(nc):
    """Reusable multi-core PJRT executor (mirrors bass_utils' axon path)."""
    import jax
    from jax.sharding import Mesh, PartitionSpec
    from jax.experimental.shard_map import shard_map
    import concourse.mybir as mybir
    from concourse.bass2jax import (install_neuronx_cc_hook, _bass_exec_p,
                                    partition_id_tensor)

    install_neuronx_cc_hook()
    partition_name = (nc.partition_id_tensor.name
                      if nc.partition_id_tensor else None)
    in_names, out_names, out_avals = [], [], []
    for alloc in nc.m.functions[0].allocations:
        if not isinstance(alloc, mybir.MemoryLocationSet):
            continue
        name = alloc.memorylocations[0].name
        if alloc.kind == "ExternalInput":
            if name != partition_name:
                in_names.append(name)
        elif alloc.kind == "ExternalOutput":
            out_names.append(name)
            out_avals.append(jax.core.ShapedArray(
                tuple(alloc.tensor_shape), mybir.dt.np(alloc.dtype)))
    n_params = len(in_names)
    all_in_names = list(in_names) + list(out_names)
    if partition_name is not None:
        all_in_names.append(partition_name)

    def _body(*args):
        operands = list(args)
        if partition_name is not None:
            operands.append(partition_id_tensor())
        outs = _bass_exec_p.bind(
            *operands,
            out_avals=tuple(out_avals),
            in_names=tuple(all_in_names),
            out_names=tuple(out_names),
            lowering_input_output_aliases=(),
            sim_require_finite=True,
            sim_require_nnan=True,
            nc=nc,
        )
        return tuple(outs)

    devices = jax.devices()[:NCORES]
    mesh = Mesh(np.asarray(devices), ("core",))
    n_outs = len(out_names)
    sharded = jax.jit(
        shard_map(_body, mesh=mesh,
                  in_specs=(PartitionSpec("core"),) * (n_params + n_outs),
                  out_specs=(PartitionSpec("core"),) * n_outs,
                  check_rep=False),
        keep_unused=True,
    )
    return sharded, in_names, out_names, out_avals


def _get_exec(niter=1):
    key = niter
    if key not in _CACHE:
        nc = _build_nc(niter)
        _CACHE[key] = _build_runner(nc)
    return _CACHE[key]


def _run(in_maps, niter=1):
    import jax
    sharded, in_names, out_names, out_avals = _get_exec(niter)
    concat_in = [np.concatenate([np.asarray(in_maps[c][n])
                                 for c in range(NCORES)], axis=0)
                 for n in in_names]
    concat_zeros = [np.zeros((NCORES * av.shape[0], *av.shape[1:]), av.dtype)
                    for av in out_avals]
    out_arrs = sharded(*concat_in, *concat_zeros)
    jax.block_until_ready(out_arrs)
    res = np.asarray(out_arrs[out_names.index("out")])
    return res.reshape(NCORES, NTOK, C)


def _host_prep(inputs):
    """Fold LN affines into weights/biases, convert dtypes, lay out
    feature-major. Pure numpy; runs once per kernel() call."""
    import ml_dtypes
    BF = ml_dtypes.bfloat16
    F8 = ml_dtypes.float8_e4m3

    f = {k: np.asarray(v, np.float32) for k, v in inputs.items()}
    g1, b1ln = f["ln1_g"], f["ln1_b"]
    g2, b2ln = f["ln2_g"], f["ln2_b"]
    Wq, Wk, Wv, Wo = f["Wq"], f["Wk"], f["Wv"], f["Wo"]
    W1, W2 = f["W1"], f["W2"]

    def fm(mat):  # [C, F] -> [P, CJ, F]
        return np.ascontiguousarray(
            mat.reshape(-1, P, mat.shape[1]).transpose(1, 0, 2))

    def headcat(Wh, scale):  # [H, C, D] (*scale per C) -> [C, H*D]
        return (Wh * scale[None, :, None]).transpose(1, 0, 2).reshape(C, C)

    wq_a = fm(headcat(Wq, g1)).astype(BF)
    wk_a = fm(headcat(Wk, g1)).astype(BF)
    wv_a = fm(headcat(Wv, g1)).astype(BF)
    wo_a = fm(Wo).astype(BF)
    w1_a = fm(W1 * g2[:, None]).astype(F8)
    # [P, FJ, C] -> chunk-pair interleaved [P, FJ//2, C, 2] for DoubleRow
    w2_a = np.ascontiguousarray(
        fm(W2).reshape(P, FJ // 2, 2, C).transpose(0, 1, 3, 2)).astype(F8)

    bq_a = np.ascontiguousarray(
        np.einsum("c,hcd->hd", b1ln, Wq).reshape(C).reshape(CJ, P).T)
    bk_a = np.ascontiguousarray(
        np.einsum("c,hcd->hd", b1ln, Wk).reshape(C).reshape(CJ, P).T)
    b1p_a = np.ascontiguousarray((f["b1"] + b2ln @ W1).reshape(FJ, P).T)
    bv = np.einsum("c,hcd->hd", b1ln, Wv).reshape(C)
    bo2 = f["bo"] + bv @ Wo
    bo2_a = np.tile(bo2[None, :], (P, 1))
    b2t_a = np.tile(f["b2"][None, :], (P, 1))

    ident = np.eye(P, dtype=BF)
    tri = np.triu(np.ones((P, P), np.float32))
    maskb = np.concatenate([tri, np.ones((P, P), np.float32), tri],
                           axis=1).astype(BF)
    onesc = np.broadcast_to(np.eye(H, dtype=np.float32)[None, :, :],
                            (P, H, H)).astype(BF)
    # -1 entries: the on-device Newton chain produces z = -1/denominator,
    # so the broadcast matmul (ind^T @ z) lands +1/denominator in psum
    ind = np.zeros((H, CJ * P), np.float32)
    for hp in range(CJ):
        for half in range(2):
            ind[2 * hp + half, hp * P + half * D:hp * P + (half + 1) * D] = -1
    indb = ind.astype(BF)

    rep = dict(wq=wq_a, wk=wk_a, wv=wv_a, wo=wo_a, w1=w1_a, w2=w2_a,
               bq=bq_a.astype(np.float32), bk=bk_a.astype(np.float32),
               b1p=b1p_a.astype(np.float32), bo2=bo2_a.astype(np.float32),
               b2t=b2t_a.astype(np.float32),
               identb=ident, maskb=maskb, onesc=onesc, indb=indb)

    x = np.ascontiguousarray(f["x"])
    in_maps = []
    for c in range(NCORES):
        m = dict(rep)
        m["x"] = x[c * B_LOC:(c + 1) * B_LOC].reshape(NTOK, C)
        in_maps.append(m)
    return in_maps


def kernel(**inputs) -> np.ndarray:
    in_maps = _host_prep(inputs)
    res = _run(in_maps, niter=1)
    return res.reshape(B, T, C)


def bench(inputs, niter=513, reps=5, floor_ns=72_400_000):
    """Estimate per-pass HW time by running the niter-looped build and
    subtracting the axon per-call RPC floor."""
    import time
    import jax as _jax
    in_maps = _host_prep(inputs)
    sharded, in_names, out_names, out_avals = _get_exec(niter)
    concat_in = [np.concatenate([np.asarray(in_maps[c][n])
                                 for c in range(NCORES)], axis=0)
                 for n in in_names]
    concat_zeros = [np.zeros((NCORES * av.shape[0], *av.shape[1:]), av.dtype)
                    for av in out_avals]
    dev_in = [_jax.device_put(a) for a in concat_in]
    dev_zeros = [_jax.device_put(a) for a in concat_zeros]
    out = sharded(*dev_in, *dev_zeros)
    _jax.block_until_ready(out)  # compile + warm
    times = []
    for _ in range(reps):
        t0 = time.perf_counter()
        out = sharded(*dev_in, *dev_zeros)
        _jax.block_until_ready(out)
        times.append(time.perf_counter() - t0)
    res = np.asarray(out[out_names.index("out")]).reshape(NCORES, NTOK, C)
    wall_ns = np.array(times) * 1e9
    per_pass = (wall_ns - floor_ns) / niter
    return res.reshape(B, T, C), per_pass, wall_ns


# revision 42
# speedup vs baseline: 1.0690x; 1.0690x over previous
# Trainium2 Bass kernel for a single pre-norm transformer block
# (LN1 -> 6-head causal self-attention -> residual -> LN2 -> 1536-wide relu MLP -> residual).
#
# Sharding: pure data-parallel over batch. B=128 sequences split 16-per-core
# across 8 NeuronCores; weights are replicated; no collectives.
#
# Design (v21):
#   - All weight preprocessing on HOST (numpy): LN gammas folded into the
#     bf16/fp8 weights, LN betas folded into fused biases (bq/bk,
#     bo2 = bo + (ln1_b@Wv)@Wo, b1' = b1 + ln2_b@W1), layouts pre-transposed
#     feature-major, constants (identity/causal-mask/one-hot/indicator)
#     shipped as inputs. The device kernel has NO preamble.
#   - Activations "feature-major" (FM): [C partitions (3x128 chunks), tokens];
#     chained matmuls need no transposes. LN runs token-major (bn_stats over
#     free dim), normalized tile PE-transposed into FM.
#   - Softmax (transposed scores, no max-subtraction):
#     denominators = one-hot-column-stationary matmuls that partition-reduce
#     each masked expT tile into row h of a [6, 384] psum group; the causal
#     mask for all 6 heads of a seq is ONE contiguous DVE multiply
#     ([triu|ones|triu] row broadcast over heads); NEGATED reciprocal via a
#     bit-trick seed + 1 Newton step (z' = (d*z+2)*z); per-head-pair
#     broadcast = ONE (-1)-indicator-stationary matmul per hp that
#     overwrites the drained attnV psum; DVE multiplies sbuf x psum.
#   - FFN entirely fp8 (e4m3) DoubleRow, with both moving operands
#     pair-interleaved (host-interleaved W2 [P, jp, C, 2]; h2 stored as
#     [P, TS, 2] pair + [P, TS] single) so DR streams byte-adjacent rows.
#   - Depth-2 software pipeline: iteration s runs supertile s and preps
#     s+2 (x DMA, LN1 chain, qkv) in its tail where DVE/Act have slack;
#     next supertile's score blocks split per-seq around the LN2 transposes
#     as PE filler; engine assignment tuned per-op (exps/yn/ar/qk-bias on
#     Act, stats/copies/divisions/z-odd on DVE, residual pre-adds on
#     GPSIMD, weights+consts DMA on the gpsimd queue, x/out on sync).
#
# Measured (NTFF profile, core 0): ~346.6us span per pass vs 471.5us for
# the session-start baseline; rel err 1.40e-2 (gate 2e-2).

import numpy as np

P = 128
B, T, C, H, D = 128, 256, 384, 6, 64
NCORES = 8
B_LOC = B // NCORES          # 16 sequences per core
NTOK = B_LOC * T             # 4096 tokens per core
TS = 2 * T                   # 512-token supertile = 2 sequences
NSUP = NTOK // TS            # 8
CJ = C // P                  # 3 chunks of the 384 model dim
FF = 4 * C                   # 1536
FJ = FF // P                 # 12 chunks of the FFN hidden dim
NTT = TS // P                # 4 token tiles per supertile
EPS = 1e-5
SCALE = D ** (-0.5)

_CACHE = {}


def _build_nc(niter=1):
    import concourse.bass as bass
    import concourse.tile as tile
    from concourse import bacc, mybir
    from contextlib import ExitStack

    F32 = mybir.dt.float32
    BF16 = mybir.dt.bfloat16
    FP8 = mybir.dt.float8e4
    DR = mybir.MatmulPerfMode.DoubleRow

    nc = bacc.Bacc("TRN2", target_bir_lowering=False, debug=False,
                   num_devices=NCORES)

    x_d = nc.dram_tensor("x", [NTOK, C], F32, kind="ExternalInput").ap()
    wq_d = nc.dram_tensor("wq", [P, CJ, C], BF16, kind="ExternalInput").ap()
    wk_d = nc.dram_tensor("wk", [P, CJ, C], BF16, kind="ExternalInput").ap()
    wv_d = nc.dram_tensor("wv", [P, CJ, C], BF16, kind="ExternalInput").ap()
    wo_d = nc.dram_tensor("wo", [P, CJ, C], BF16, kind="ExternalInput").ap()
    w1_d = nc.dram_tensor("w1", [P, CJ, FF], FP8, kind="ExternalInput").ap()
    # w2 pre-interleaved on host: [P, pair jp, C, 2] so the DoubleRow moving
    # operand reads byte-adjacent chunk pairs (full 2x fp8 stream rate)
    w2_d = nc.dram_tensor("w2", [P, FJ // 2, C, 2], FP8, kind="ExternalInput").ap()
    bq_d = nc.dram_tensor("bq", [P, CJ], F32, kind="ExternalInput").ap()
    bk_d = nc.dram_tensor("bk", [P, CJ], F32, kind="ExternalInput").ap()
    b1p_d = nc.dram_tensor("b1p", [P, FJ], F32, kind="ExternalInput").ap()
    bo2_d = nc.dram_tensor("bo2", [P, C], F32, kind="ExternalInput").ap()
    b2t_d = nc.dram_tensor("b2t", [P, C], F32, kind="ExternalInput").ap()
    ident_d = nc.dram_tensor("identb", [P, P], BF16, kind="ExternalInput").ap()
    mask_d = nc.dram_tensor("maskb", [P, CJ * P], BF16, kind="ExternalInput").ap()
    ones_d = nc.dram_tensor("onesc", [P, H, H], BF16, kind="ExternalInput").ap()
    ind_d = nc.dram_tensor("indb", [H, CJ * P], BF16, kind="ExternalInput").ap()
    out_d = nc.dram_tensor("out", [NTOK, C], F32, kind="ExternalOutput").ap()

    Exp = mybir.ActivationFunctionType.Exp
    Relu = mybir.ActivationFunctionType.Relu
    Ident = mybir.ActivationFunctionType.Identity
    I32 = mybir.dt.int32
    ADD = mybir.AluOpType.add
    MULT = mybir.AluOpType.mult
    MAX = mybir.AluOpType.max
    SHR = mybir.AluOpType.logical_shift_right

    with tile.TileContext(nc) as tc, ExitStack() as ctx:
        consts = ctx.enter_context(tc.tile_pool(name="consts", bufs=1))
        wpool = ctx.enter_context(tc.tile_pool(name="weights", bufs=1))
        xpool = ctx.enter_context(tc.tile_pool(name="xln", bufs=12))
        ps_big = ctx.enter_context(tc.tile_pool(name="psbig", bufs=3, space="PSUM"))
        ps_tr = ctx.enter_context(tc.tile_pool(name="pstr", bufs=2, space="PSUM"))
        ps_dn = ctx.enter_context(tc.tile_pool(name="psdn", bufs=1, space="PSUM"))
        ps_at = ctx.enter_context(tc.tile_pool(name="psat", bufs=2, space="PSUM"))

        # ----------- constants + weights: DMAs on the GPSIMD queue ------
        # (x loads / out stores own the Sync queue; Act queue stays clean
        # for the first LN normalize; gpsimd is idle until mid-supertile-0
        # so the ~15 descriptor issues cost nothing)
        ident_bf = consts.tile([P, P], BF16, tag="identbf")
        nc.gpsimd.dma_start(ident_bf[:], ident_d)
        maskf = consts.tile([P, CJ * P], BF16, tag="mask")
        nc.gpsimd.dma_start(maskf[:], mask_d)
        # onesh[:, h, :] is the one-hot-column stationary that routes head
        # h's partition-reduction into psum row h (matmul psum base must be
        # 0/32/64, so all heads accumulate into one base-0 [H, T] group)
        onesh = consts.tile([P, H, H], BF16, tag="onesh")
        nc.gpsimd.dma_start(onesh[:], ones_d)
        ind6 = consts.tile([H, CJ * P], BF16, tag="ind6")
        nc.gpsimd.dma_start(ind6[:], ind_d)
        bq = consts.tile([P, CJ], F32, tag="bq")
        nc.gpsimd.dma_start(bq[:], bq_d)
        bk = consts.tile([P, CJ], F32, tag="bk")
        nc.gpsimd.dma_start(bk[:], bk_d)
        b1p = consts.tile([P, FJ], F32, tag="b1p")
        nc.gpsimd.dma_start(b1p[:], b1p_d)
        bo2_bc = consts.tile([P, C], F32, tag="bo2_bc")
        nc.gpsimd.dma_start(bo2_bc[:], bo2_d)
        b2_bc = consts.tile([P, C], F32, tag="b2_bc")
        nc.gpsimd.dma_start(b2_bc[:], b2t_d)

        wq = wpool.tile([P, CJ, C], BF16, tag="wqb")
        nc.gpsimd.dma_start(wq[:], wq_d)
        wk = wpool.tile([P, CJ, C], BF16, tag="wkb")
        nc.gpsimd.dma_start(wk[:], wk_d)
        wv = wpool.tile([P, CJ, C], BF16, tag="wvb")
        nc.gpsimd.dma_start(wv[:], wv_d)
        wo = wpool.tile([P, CJ, C], BF16, tag="wob")
        nc.gpsimd.dma_start(wo[:], wo_d)
        w1 = wpool.tile([P, CJ, FF], FP8, tag="w1b")
        nc.gpsimd.dma_start(w1[:], w1_d)
        w2 = wpool.tile([P, FJ // 2, C, 2], FP8, tag="w2b")
        nc.gpsimd.dma_start(w2[:], w2_d)

        def load_x(s):
            tok0 = s * TS
            x_ts = []
            for ti in range(NTT):
                x_t = xpool.tile([P, C], F32, tag="x")
                nc.sync.dma_start(
                    x_t[:], x_d[tok0 + ti * P: tok0 + (ti + 1) * P, :])
                x_ts.append(x_t)
            return x_ts

        # ---------------- layernorm helpers ----------------
        spool = ctx.enter_context(tc.tile_pool(name="stats", bufs=6))
        ynpool = ctx.enter_context(tc.tile_pool(name="yn", bufs=8))

        def ln_stats_pre(src_tiles):
            """bn_stats per tile (each depends only on its x DMA)."""
            sts = []
            for ti in range(NTT):
                st = spool.tile([P, 6], F32, tag="bn")
                nc.vector.bn_stats(st[:], src_tiles[ti][:])
                sts.append(st)
            return sts

        def ln_stats_post(sts):
            """Aggregate + rstd Newton chain (emitted later so latency-
            critical attention DVE ops aren't queued behind it)."""
            mv4 = spool.tile([P, NTT, 2], F32, tag="mv")
            rstd4 = spool.tile([P, NTT], F32, tag="rstd")
            for ti in range(NTT):
                nc.vector.bn_aggr(mv4[:, ti, :], sts[ti][:])
            # rstd = rsqrt(var + eps): int32 seed + 2 Newton steps (no tables)
            veps = spool.tile([P, NTT], F32, tag="veps")
            nc.vector.tensor_scalar_add(veps[:], mv4[:, :, 1], EPS)
            iv = spool.tile([P, NTT], I32, tag="ivh")
            nc.vector.tensor_scalar(iv[:], veps[:].bitcast(I32), 1, None, op0=SHR)
            nc.vector.tensor_scalar(iv[:], iv[:], -1, 0x5F3759DF, op0=MULT, op1=ADD)
            tn = spool.tile([P, NTT], F32, tag="tnh")
            yv = iv[:].bitcast(F32)
            # 1 Newton step: ~0.1% rstd error, well under the bf16 yn output
            for it in range(1):
                nc.vector.tensor_tensor(tn[:], yv, yv, op=MULT)
                nc.vector.scalar_tensor_tensor(tn[:], tn[:], -0.5, veps[:],
                                               op0=MULT, op1=MULT)
                nc.vector.scalar_tensor_tensor(yv, tn[:], 1.5, yv,
                                               op0=ADD, op1=MULT)
            nc.vector.tensor_copy(rstd4[:], yv)
            nbias = spool.tile([P, NTT], F32, tag="nb")
            nc.vector.scalar_tensor_tensor(nbias[:], mv4[:, :, 0], -1.0,
                                           rstd4[:], op0=MULT, op1=MULT)
            return rstd4, nbias

        def ln_apply(src_tiles, stats, dst_fm):
            """Normalize token-major (Act) + PE transpose to FM + DVE copy."""
            rstd4, nbias = stats
            for ti in range(NTT):
                yn = ynpool.tile([P, C], BF16, tag="yn")
                nc.scalar.activation(yn[:], src_tiles[ti][:], Ident,
                                     bias=nbias[:, ti:ti + 1],
                                     scale=rstd4[:, ti:ti + 1])
                pst = ps_tr.tile([P, C], BF16, tag="tr")
                for j in range(CJ):
                    nc.tensor.transpose(pst[:, j * P:(j + 1) * P],
                                        yn[:, j * P:(j + 1) * P], ident_bf[:])
                nc.vector.tensor_copy(
                    dst_fm[:, :, ti * P:(ti + 1) * P],
                    pst[:].rearrange("p (j t) -> p j t", j=CJ))

        def ln_apply_pair(src_tiles, stats, dstp, dsts):
            """Like ln_apply but writes chunks 0,1 pair-interleaved
            ([P, TS, 2]) + chunk 2 separate, so the FFN1 DoubleRow moving
            operand reads byte-adjacent chunk pairs."""
            rstd4, nbias = stats
            for ti in range(NTT):
                yn = ynpool.tile([P, C], BF16, tag="yn")
                nc.scalar.activation(yn[:], src_tiles[ti][:], Ident,
                                     bias=nbias[:, ti:ti + 1],
                                     scale=rstd4[:, ti:ti + 1])
                pst = ps_tr.tile([P, C], BF16, tag="tr")
                for j in range(CJ):
                    nc.tensor.transpose(pst[:, j * P:(j + 1) * P],
                                        yn[:, j * P:(j + 1) * P], ident_bf[:])
                nc.vector.tensor_copy(
                    dstp[:, ti * P:(ti + 1) * P, :],
                    pst[:, 0:2 * P].rearrange("p (two t) -> p t two", two=2))
                nc.vector.tensor_copy(dsts[:, ti * P:(ti + 1) * P],
                                      pst[:, 2 * P:3 * P])

        # ---------------- pools for the main phases ----------------
        hpool = ctx.enter_context(tc.tile_pool(name="hfm", bufs=2))
        h2pool = ctx.enter_context(tc.tile_pool(name="h2fm", bufs=2))
        qkpool = ctx.enter_context(tc.tile_pool(name="qk", bufs=6))
        vpool = ctx.enter_context(tc.tile_pool(name="vton", bufs=12))
        xbpool = ctx.enter_context(tc.tile_pool(name="xbo", bufs=6))
        o1pool = ctx.enter_context(tc.tile_pool(name="o1res", bufs=6))
        obpool = ctx.enter_context(tc.tile_pool(name="o1b2", bufs=6))
        apool = ctx.enter_context(tc.tile_pool(name="attnfm", bufs=2))
        epool = ctx.enter_context(tc.tile_pool(name="expT", bufs=4))
        arpool = ctx.enter_context(tc.tile_pool(name="attnraw", bufs=8))
        zpool = ctx.enter_context(tc.tile_pool(name="zfm", bufs=1))
        ypool = ctx.enter_context(tc.tile_pool(name="yout", bufs=3))

        def ln_stats(src_tiles):
            return ln_stats_post(ln_stats_pre(src_tiles))

        def ln1_first(x_ts):
            """Supertile-0 LN1 with per-tile serial chains: tile 0's
            normalize/transpose starts as soon as ITS stats are done
            instead of after all four bn_stats (startup critical path)."""
            h_fm = hpool.tile([P, CJ, TS], BF16, tag="hfm")
            for ti in range(NTT):
                st = spool.tile([P, 6], F32, tag="bn")
                nc.vector.bn_stats(st[:], x_ts[ti][:])
                mv = spool.tile([P, 2], F32, tag="mv1")
                nc.vector.bn_aggr(mv[:], st[:])
                veps = spool.tile([P, 1], F32, tag="veps1")
                nc.vector.tensor_scalar_add(veps[:], mv[:, 1:2], EPS)
                iv = spool.tile([P, 1], I32, tag="iv1")
                nc.vector.tensor_scalar(iv[:], veps[:].bitcast(I32), 1, None,
                                        op0=SHR)
                nc.vector.tensor_scalar(iv[:], iv[:], -1, 0x5F3759DF,
                                        op0=MULT, op1=ADD)
                tn = spool.tile([P, 1], F32, tag="tn1")
                yv = iv[:].bitcast(F32)
                nc.vector.tensor_tensor(tn[:], yv, yv, op=MULT)
                nc.vector.scalar_tensor_tensor(tn[:], tn[:], -0.5, veps[:],
                                               op0=MULT, op1=MULT)
                nc.vector.scalar_tensor_tensor(yv, tn[:], 1.5, yv,
                                               op0=ADD, op1=MULT)
                nbias = spool.tile([P, 1], F32, tag="nb1")
                nc.vector.scalar_tensor_tensor(nbias[:], mv[:, 0:1], -1.0,
                                               yv, op0=MULT, op1=MULT)
                yn = ynpool.tile([P, C], BF16, tag="yn")
                nc.scalar.activation(yn[:], x_ts[ti][:], Ident,
                                     bias=nbias[:], scale=yv)
                pst = ps_tr.tile([P, C], BF16, tag="tr")
                for j in range(CJ):
                    nc.tensor.transpose(pst[:, j * P:(j + 1) * P],
                                        yn[:, j * P:(j + 1) * P], ident_bf[:])
                nc.vector.tensor_copy(
                    h_fm[:, :, ti * P:(ti + 1) * P],
                    pst[:].rearrange("p (j t) -> p j t", j=CJ))
            return h_fm

        def qkv_phase(h_fm):
            q_fm = qkpool.tile([P, CJ, TS], BF16, tag="qk")
            k_fm = qkpool.tile([P, CJ, TS], BF16, tag="qk")
            for wt, bt, dst in ((wq, bq, q_fm), (wk, bk, k_fm)):
                for f in range(CJ):
                    ps = ps_big.tile([P, TS], F32, tag="big")
                    for j in range(CJ):
                        nc.tensor.matmul(
                            ps[:], lhsT=wt[:, j, f * P:(f + 1) * P],
                            rhs=h_fm[:, j, :],
                            start=(j == 0), stop=(j == CJ - 1))
                    nc.scalar.activation(dst[:, f, :], ps[:], Ident,
                                         bias=bt[:, f:f + 1])
            v_ts = []
            for ti in range(NTT):
                ps = ps_big.tile([P, C], F32, tag="big")
                for j in range(CJ):
                    nc.tensor.matmul(
                        ps[:], lhsT=h_fm[:, j, ti * P:(ti + 1) * P],
                        rhs=wv[:, j, :],
                        start=(j == 0), stop=(j == CJ - 1))
                v_t = vpool.tile([P, C], BF16, tag="v")
                # DVE, not Act: the Act queue is the constraint in the
                # qkv->ln2 window
                nc.vector.tensor_copy(v_t[:], ps[:])
                v_ts.append(v_t)
            return q_fm, k_fm, v_ts

        def score_block(q_fm, k_fm, e_seq, seq, h, pool=None):
            """Scores + exp for one (seq, head) block into e_seq[:, h, :].
            narrow layout: cols 0:256 = [k 0:128 x q 0:256], cols 256:384 =
            [k 128:256 x q 128:256]."""
            t0 = seq * T
            hp, hh = h // 2, h % 2
            pr = slice(hh * D, (hh + 1) * D)
            if pool is None:
                ps_sc = ps_big.tile([P, 3 * P], F32, tag="big")
            else:
                ps_sc = pool.tile([P, 3 * P], F32, tag="at")
            nc.tensor.matmul(ps_sc[:, 0:T],
                             lhsT=k_fm[pr, hp, t0:t0 + P],
                             rhs=q_fm[pr, hp, t0:t0 + T],
                             start=True, stop=True)
            nc.tensor.matmul(ps_sc[:, T:T + P],
                             lhsT=k_fm[pr, hp, t0 + P:t0 + T],
                             rhs=q_fm[pr, hp, t0 + P:t0 + T],
                             start=True, stop=True)
            nc.scalar.activation(e_seq[:, h, :], ps_sc[:], Exp, scale=SCALE)

        def mask_seq(e_seq):
            """Mask all 6 heads in one contiguous-inner-dim DVE multiply:
            maskf = [triu | ones | triu] broadcast over heads via a
            0-stride dim (contiguous 384-wide rows keep DVE at full rate;
            the middle block multiplies by 1)."""
            ev = e_seq[:]
            mk = maskf[:]
            mbc = bass.AP(tensor=mk.tensor, offset=mk.offset,
                          ap=[list(mk.ap[0]), [0, H], [1, CJ * P]])
            nc.vector.tensor_tensor(ev, ev, mbc, op=MULT)

        def attn_scores_seq(q_fm, k_fm, seq, pool=None, heads=None,
                            e_seq=None, mask=True):
            """Scores+exp(+mask) for head blocks of one seq."""
            if e_seq is None:
                e_seq = epool.tile([P, H, 3 * P], BF16, tag="e")
            for h in (heads if heads is not None else range(H)):
                score_block(q_fm, k_fm, e_seq, seq, h, pool=pool)
            if mask:
                mask_seq(e_seq)
                return [e_seq[:, h, :] for h in range(H)]
            return e_seq

        def attn_scores(q_fm, k_fm):
            return [attn_scores_seq(q_fm, k_fm, 0),
                    attn_scores_seq(q_fm, k_fm, 1)]

        def attention_seq(exps_2, v_ts, attn_fm, seq):
                t0 = seq * T
                v0, v1 = v_ts[2 * seq], v_ts[2 * seq + 1]
                exps = exps_2[seq]  # noqa: kept names for the body below
                # denominators: partition-reduce each masked expT into row h
                # of dn via the one-hot-column stationary (rows != h get +0);
                # all 6 matmuls accumulate into one base-0 psum group.
                # dn cols 0:256 = keys 0:128 over q 0:256; cols 256:384 =
                # keys 128:256 over q 128:256 (folded below on DVE).
                dn = ps_dn.tile([H, 3 * P], F32, tag="dn")
                for h in range(H):
                    nc.tensor.matmul(dn[:],
                                     lhsT=onesh[:, h, :], rhs=exps[h][:],
                                     start=(h == 0), stop=(h == H - 1))
                # fold + NEGATED reciprocal via bit-trick seed + 1 Newton
                # step in z = -1/d space (z' = (d*z + 2)*z); the indicator
                # matrix carries -1 entries so the broadcast flips the sign.
                dsb = spool.tile([H, 3 * P], F32, tag="dsb")
                nc.vector.tensor_copy(dsb[:], dn[:])
                nc.vector.tensor_tensor(dsb[:, P:T], dsb[:, P:T],
                                        dsb[:, T:3 * P], op=ADD)
                zi = spool.tile([H, T], I32, tag="zi")
                nc.vector.tensor_scalar(zi[:], dsb[:, 0:T].bitcast(I32),
                                        -1, 0xFEF311C3 - (1 << 32),
                                        op0=MULT, op1=ADD)
                zf = zi[:].bitcast(F32)
                tn2 = spool.tile([H, T], F32, tag="tn2")
                nc.vector.tensor_tensor(tn2[:], dsb[:, 0:T], zf, op=MULT)
                recip = spool.tile([H, T], BF16, tag="recip")
                with nc.allow_low_precision(reason="bf16 softmax recip"):
                    nc.vector.scalar_tensor_tensor(recip[:], tn2[:], 2.0, zf,
                                                   op0=ADD, op1=MULT)
                # attnV: two [P, 512] psum tiles hold hp0|hp1 and hp2|M0;
                # M1/M2 overwrite the drained hp0/hp1 regions.
                psA = ps_at.tile([P, TS], F32, tag="at")
                psB = ps_at.tile([P, TS], F32, tag="at")
                regions = [(psA, 0), (psA, T), (psB, 0)]
                mregions = [(psB, T), (psA, 0), (psA, T)]
                ars = []
                for hp in range(CJ):
                    ps_a, c0 = regions[hp]
                    for hh in range(2):
                        h = 2 * hp + hh
                        po = slice(hh * D, (hh + 1) * D)
                        nc.tensor.matmul(ps_a[po, c0:c0 + T],
                                         lhsT=v0[:, h * D:(h + 1) * D],
                                         rhs=exps[h][:, 0:T],
                                         start=True, stop=False)
                        nc.tensor.matmul(ps_a[po, c0 + P:c0 + T],
                                         lhsT=v1[:, h * D:(h + 1) * D],
                                         rhs=exps[h][:, T:T + P],
                                         start=False, stop=True)
                    ar = arpool.tile([P, T], BF16, tag="ar")
                    # Act, not DVE: Act idles during attention and this
                    # keeps the psum-drain chain off the loaded DVE queue
                    nc.scalar.activation(ar[:], ps_a[:, c0:c0 + T], Ident)
                    ars.append(ar)
                # recip row-broadcast per head-pair psum block, then divide
                for hp in range(CJ):
                    ps_m, m0 = mregions[hp]
                    nc.tensor.matmul(ps_m[:, m0:m0 + T],
                                     lhsT=ind6[:, hp * P:(hp + 1) * P],
                                     rhs=recip[:], start=True, stop=True)
                    nc.vector.tensor_tensor(attn_fm[:, hp, t0:t0 + T],
                                            ars[hp][:], ps_m[:, m0:m0 + T],
                                            op=MULT)

        def wo_tile(attn_fm, x_ts, ti, o1_ts, ob_ts):
            xb = xbpool.tile([P, C], F32, tag="xb")
            nc.gpsimd.tensor_tensor(xb[:], x_ts[ti][:], bo2_bc[:], op=ADD)
            # ps_at (free after attention), so qk never waits on wo drains
            ps = ps_at.tile([P, C], F32, tag="at")
            for j in range(CJ):
                nc.tensor.matmul(
                    ps[:], lhsT=attn_fm[:, j, ti * P:(ti + 1) * P],
                    rhs=wo[:, j, :],
                    start=(j == 0), stop=(j == CJ - 1))
            o1 = o1pool.tile([P, C], F32, tag="o1")
            nc.vector.tensor_tensor(o1[:], ps[:], xb[:], op=ADD)
            o1_ts.append(o1)
            ob = obpool.tile([P, C], F32, tag="ob")
            nc.gpsimd.tensor_tensor(ob[:], o1[:], b2_bc[:], op=ADD)
            ob_ts.append(ob)

        def attn_wo_phase(exps_2, v_ts, x_ts):
            attn_fm = apool.tile([P, CJ, TS], BF16, tag="attn")
            o1_ts, ob_ts, sts = [], [], []
            attention_seq(exps_2, v_ts, attn_fm, 0)
            attention_seq(exps_2, v_ts, attn_fm, 1)
            for ti in range(NTT):
                wo_tile(attn_fm, x_ts, ti, o1_ts, ob_ts)
                # LN2 stats per tile right behind its o1 add: shortens the
                # serial o1 -> stats -> yn -> transpose chain
                st = spool.tile([P, 6], F32, tag="bn")
                nc.vector.bn_stats(st[:], o1_ts[ti][:])
                sts.append(st)
            return o1_ts, ob_ts, sts

        def ffn_phase(h2p, h2s, ob_ts, tok0, qk_next=None):
            """FFN for supertile s; when qk_next is given, the NEXT
            supertile's 12 score blocks interleave into the f-chunk loop
            (one per chunk) so their exps stream on Act alongside the z
            writes instead of piling up after them."""
            exps_next = None
            e_cur = None
            if qk_next is not None:
                q_fm, k_fm = qk_next
                exps_next = []
            z_fm = zpool.tile([P, FJ, TS], FP8, tag="z")
            h2pv = h2p[:].rearrange("p t two -> p two t")
            for f in range(FJ):
                ps = ps_big.tile([P, TS], F32, tag="big")
                if f < 2:
                    # token-split: these start after only 2 h2 tiles land,
                    # instead of waiting for the whole supertile's copies
                    for half in range(2):
                        sl = slice(half * T, (half + 1) * T)
                        nc.tensor.matmul(
                            ps[:, sl], lhsT=w1[:, 0:2, f * P:(f + 1) * P],
                            rhs=h2pv[:, :, sl], perf_mode=DR,
                            start=True, stop=False)
                        nc.tensor.matmul(
                            ps[:, sl], lhsT=w1[:, 2, f * P:(f + 1) * P],
                            rhs=h2s[:, sl],
                            start=False, stop=True)
                else:
                    nc.tensor.matmul(
                        ps[:], lhsT=w1[:, 0:2, f * P:(f + 1) * P],
                        rhs=h2pv, perf_mode=DR,
                        start=True, stop=False)
                    nc.tensor.matmul(
                        ps[:], lhsT=w1[:, 2, f * P:(f + 1) * P],
                        rhs=h2s[:],
                        start=False, stop=True)
                if f % 2 == 0:
                    nc.scalar.activation(z_fm[:, f, :], ps[:], Relu,
                                         bias=b1p[:, f:f + 1])
                else:
                    nc.vector.tensor_scalar(z_fm[:, f, :], ps[:],
                                            b1p[:, f:f + 1], 0.0,
                                            op0=ADD, op1=MAX)
                if qk_next is not None:
                    seq, h = divmod(f, H)
                    if h == 0:
                        e_cur = epool.tile([P, H, 3 * P], BF16, tag="e")
                    score_block(q_fm, k_fm, e_cur, seq, h)
                    if h == H - 1:
                        mask_seq(e_cur)
                        exps_next.append([e_cur[:, h2, :] for h2 in range(H)])
            for ti in range(NTT):
                ps = ps_big.tile([P, C], F32, tag="big")
                for jp in range(FJ // 2):
                    nc.tensor.matmul(
                        ps[:], lhsT=z_fm[:, 2 * jp:2 * jp + 2,
                                         ti * P:(ti + 1) * P],
                        rhs=w2[:, jp].rearrange("p c two -> p two c"),
                        perf_mode=DR,
                        start=(jp == 0), stop=(jp == FJ // 2 - 1))
                y_t = ypool.tile([P, C], F32, tag="y")
                nc.vector.tensor_tensor(y_t[:], ps[:], ob_ts[ti][:], op=ADD)
                nc.sync.dma_start(
                    out_d[tok0 + ti * P: tok0 + (ti + 1) * P, :], y_t[:])
            return exps_next

        def main_pass(_iv=None):
            # depth-2 software pipeline: iteration s runs the block for
            # supertile s and PREPS supertile s+2 (x DMA, LN1 chain, qkv) in
            # its tail, where DVE/Act have slack -- the serial LN1 chain is
            # never on the critical path. exps(s+1) also stream at the tail.
            x_pre = {0: load_x(0)}
            h0 = ln1_first(x_pre[0])
            qkv_pre = {0: qkv_phase(h0)}
            e0_0 = attn_scores_seq(qkv_pre[0][0], qkv_pre[0][1], 0)
            x_pre[1] = load_x(1)
            h1 = hpool.tile([P, CJ, TS], BF16, tag="hfm")
            ln_apply(x_pre[1], ln_stats(x_pre[1]), h1)
            e0_1 = attn_scores_seq(qkv_pre[0][0], qkv_pre[0][1], 1)
            qkv_pre[1] = qkv_phase(h1)
            exps_cur = [e0_0, e0_1]
            for s in range(NSUP):
                o1_ts, ob_ts, sts2 = attn_wo_phase(exps_cur, qkv_pre[s][2],
                                                   x_pre[s])
                st2 = ln_stats_post(sts2)
                if s + 1 < NSUP:
                    # seq0 of the next supertile's scores: PE filler between
                    # wo and the LN2 transposes (hides the serial LN2 chain)
                    e0_next = attn_scores_seq(qkv_pre[s + 1][0],
                                              qkv_pre[s + 1][1], 0)
                h2p = h2pool.tile([P, TS, 2], FP8, tag="h2p")
                h2s = h2pool.tile([P, TS], FP8, tag="h2s")
                ln_apply_pair(o1_ts, st2, h2p, h2s)
                if s + 1 < NSUP:
                    # seq1 scores: PE filler between the LN2 transposes and
                    # ffn1 (hides the h2 psum->sbuf copy latency); psums from
                    # ps_at (idle here) so ffn1 owns ps_big uncontended
                    e1_next = attn_scores_seq(qkv_pre[s + 1][0],
                                              qkv_pre[s + 1][1], 1,
                                              pool=ps_at)
                    exps_cur = [e0_next, e1_next]
                ffn_phase(h2p, h2s, ob_ts, s * TS)
                if s + 2 < NSUP:
                    x_pre[s + 2] = load_x(s + 2)
                    st = ln_stats(x_pre[s + 2])
                    h_n = hpool.tile([P, CJ, TS], BF16, tag="hfm")
                    ln_apply(x_pre[s + 2], st, h_n)
                    qkv_pre[s + 2] = qkv_phase(h_n)
                    del x_pre[s], qkv_pre[s]

        if niter == 1:
            main_pass()
        else:
            with tc.For_i(0, niter, 1) as iv:
                main_pass(iv)

    nc.compile()
    return nc


def _build_runner(nc):
    """Reusable multi-core PJRT executor (mirrors bass_utils' axon path)."""
    import jax
    from jax.sharding import Mesh, PartitionSpec
    from jax.experimental.shard_map import shard_map
    import concourse.mybir as mybir
    from concourse.bass2jax import (install_neuronx_cc_hook, _bass_exec_p,
                                    partition_id_tensor)

    install_neuronx_cc_hook()
    partition_name = (nc.partition_id_tensor.name
                      if nc.partition_id_tensor else None)
    in_names, out_names, out_avals = [], [], []
    for alloc in nc.m.functions[0].allocations:
        if not isinstance(alloc, mybir.MemoryLocationSet):
            continue
        name = alloc.memorylocations[0].name
        if alloc.kind == "ExternalInput":
            if name != partition_name:
                in_names.append(name)
        elif alloc.kind == "ExternalOutput":
            out_names.append(name)
            out_avals.append(jax.core.ShapedArray(
                tuple(alloc.tensor_shape), mybir.dt.np(alloc.dtype)))
    n_params = len(in_names)
    all_in_names = list(in_names) + list(out_names)
    if partition_name is not None:
        all_in_names.append(partition_name)

    def _body(*args):
        operands = list(args)
        if partition_name is not None:
            operands.append(partition_id_tensor())
        outs = _bass_exec_p.bind(
            *operands,
            out_avals=tuple(out_avals),
            in_names=tuple(all_in_names),
            out_names=tuple(out_names),
            lowering_input_output_aliases=(),
            sim_require_finite=True,
            sim_require_nnan=True,
            nc=nc,
        )
        return tuple(outs)

    devices = jax.devices()[:NCORES]
    mesh = Mesh(np.asarray(devices), ("core",))
    n_outs = len(out_names)
    sharded = jax.jit(
        shard_map(_body, mesh=mesh,
                  in_specs=(PartitionSpec("core"),) * (n_params + n_outs),
                  out_specs=(PartitionSpec("core"),) * n_outs,
                  check_rep=False),
        keep_unused=True,
    )
    return sharded, in_names, out_names, out_avals


def _get_exec(niter=1):
    key = niter
    if key not in _CACHE:
        nc = _build_nc(niter)
        _CACHE[key] = _build_runner(nc)
    return _CACHE[key]


def _run(in_maps, niter=1):
    import jax
    sharded, in_names, out_names, out_avals = _get_exec(niter)
    concat_in = [np.concatenate([np.asarray(in_maps[c][n])
                                 for c in range(NCORES)], axis=0)
                 for n in in_names]
    concat_zeros = [np.zeros((NCORES * av.shape[0], *av.shape[1:]), av.dtype)
                    for av in out_avals]
    out_arrs = sharded(*concat_in, *concat_zeros)
    jax.block_until_ready(out_arrs)
    res = np.asarray(out_arrs[out_names.index("out")])
    return res.reshape(NCORES, NTOK, C)


def _host_prep(inputs):
    """Fold LN affines into weights/biases, convert dtypes, lay out
    feature-major. Pure numpy; runs once per kernel() call."""
    import ml_dtypes
    BF = ml_dtypes.bfloat16
    F8 = ml_dtypes.float8_e4m3

    f = {k: np.asarray(v, np.float32) for k, v in inputs.items()}
    g1, b1ln = f["ln1_g"], f["ln1_b"]
    g2, b2ln = f["ln2_g"], f["ln2_b"]
    Wq, Wk, Wv, Wo = f["Wq"], f["Wk"], f["Wv"], f["Wo"]
    W1, W2 = f["W1"], f["W2"]

    def fm(mat):  # [C, F] -> [P, CJ, F]
        return np.ascontiguousarray(
            mat.reshape(-1, P, mat.shape[1]).transpose(1, 0, 2))

    def headcat(Wh, scale):  # [H, C, D] (*scale per C) -> [C, H*D]
        return (Wh * scale[None, :, None]).transpose(1, 0, 2).reshape(C, C)

    wq_a = fm(headcat(Wq, g1)).astype(BF)
    wk_a = fm(headcat(Wk, g1)).astype(BF)
    wv_a = fm(headcat(Wv, g1)).astype(BF)
    wo_a = fm(Wo).astype(BF)
    w1_a = fm(W1 * g2[:, None]).astype(F8)
    # [P, FJ, C] -> chunk-pair interleaved [P, FJ//2, C, 2] for DoubleRow
    w2_a = np.ascontiguousarray(
        fm(W2).reshape(P, FJ // 2, 2, C).transpose(0, 1, 3, 2)).astype(F8)

    bq_a = np.ascontiguousarray(
        np.einsum("c,hcd->hd", b1ln, Wq).reshape(C).reshape(CJ, P).T)
    bk_a = np.ascontiguousarray(
        np.einsum("c,hcd->hd", b1ln, Wk).reshape(C).reshape(CJ, P).T)
    b1p_a = np.ascontiguousarray((f["b1"] + b2ln @ W1).reshape(FJ, P).T)
    bv = np.einsum("c,hcd->hd", b1ln, Wv).reshape(C)
    bo2 = f["bo"] + bv @ Wo
    bo2_a = np.tile(bo2[None, :], (P, 1))
    b2t_a = np.tile(f["b2"][None, :], (P, 1))

    ident = np.eye(P, dtype=BF)
    tri = np.triu(np.ones((P, P), np.float32))
    maskb = np.concatenate([tri, np.ones((P, P), np.float32), tri],
                           axis=1).astype(BF)
    onesc = np.broadcast_to(np.eye(H, dtype=np.float32)[None, :, :],
                            (P, H, H)).astype(BF)
    # -1 entries: the on-device Newton chain produces z = -1/denominator,
    # so the broadcast matmul (ind^T @ z) lands +1/denominator in psum
    ind = np.zeros((H, CJ * P), np.float32)
    for hp in range(CJ):
        for half in range(2):
            ind[2 * hp + half, hp * P + half * D:hp * P + (half + 1) * D] = -1
    indb = ind.astype(BF)

    rep = dict(wq=wq_a, wk=wk_a, wv=wv_a, wo=wo_a, w1=w1_a, w2=w2_a,
               bq=bq_a.astype(np.float32), bk=bk_a.astype(np.float32),
               b1p=b1p_a.astype(np.float32), bo2=bo2_a.astype(np.float32),
               b2t=b2t_a.astype(np.float32),
               identb=ident, maskb=maskb, onesc=onesc, indb=indb)

    x = np.ascontiguousarray(f["x"])
    in_maps = []
    for c in range(NCORES):
        m = dict(rep)
        m["x"] = x[c * B_LOC:(c + 1) * B_LOC].reshape(NTOK, C)
        in_maps.append(m)
    return in_maps


def kernel(**inputs) -> np.ndarray:
    in_maps = _host_prep(inputs)
    res = _run(in_maps, niter=1)
    return res.reshape(B, T, C)


def bench(inputs, niter=513, reps=5, floor_ns=72_400_000):
    """Estimate per-pass HW time by running the niter-looped build and
    subtracting the axon per-call RPC floor."""
    import time
    import jax as _jax
    in_maps = _host_prep(inputs)
    sharded, in_names, out_names, out_avals = _get_exec(niter)
    concat_in = [np.concatenate([np.asarray(in_maps[c][n])
                                 for c in range(NCORES)], axis=0)
                 for n in in_names]
    concat_zeros = [np.zeros((NCORES * av.shape[0], *av.shape[1:]), av.dtype)
                    for av in out_avals]
    dev_in = [_jax.device_put(a) for a in concat_in]
    dev_zeros = [_jax.device_put(a) for a in concat_zeros]
    out = sharded(*dev_in, *dev_zeros)
    _jax.block_until_ready(out)  # compile + warm
    times = []
    for _ in range(reps):
        t0 = time.perf_counter()
        out = sharded(*dev_in, *dev_zeros)
        _jax.block_until_ready(out)
        times.append(time.perf_counter() - t0)
    res = np.asarray(out[out_names.index("out")]).reshape(NCORES, NTOK, C)
    wall_ns = np.array(times) * 1e9
    per_pass = (wall_ns - floor_ns) / niter
    return res.reshape(B, T, C), per_pass, wall_ns
